# revision 2
# baseline (speedup 1.0000x reference)
"""ContextWeaver: context[i, j] = relu(sum_{k,d} node[i,k,d] * edge[j,k,d]), diag zeroed.

Strategy (8 NeuronCores, SPMD):
  - Shard node rows 8-way (1024 rows/core); replicate edge^T per core with a
    per-core column rotation of c*1024 so the diagonal block lands at local
    columns [m*128, (m+1)*128) of every 128-row strip -- the instruction
    stream is identical on all cores and diagonal masking is fully static.
  - Contraction dim is 64 (= K*D); pack two independent 64-row matmuls into
    the 128x128 PE array with tile_position row tiling: partitions 0-63
    compute local columns [0, 4096), partitions 64-127 compute [4096, 8192).
  - The kernel is output-DMA bound (256 MB result). Inputs and outputs are
    fp16 on device: matmul in fp16 (fp32 PSUM accumulate), relu converts
    PSUM fp32 -> fp16 strips, output DMA moves half the bytes of fp32. The
    host upcasts to fp32 on unshard; total rounding error ~5e-4 relative,
    far inside the 2e-2 gate.
  - PSUM -> SBUF relu split between ScalarE (Relu activation) and VectorE
    (tensor_scalar_max); per-strip [128, 8192] staging; 1 MiB output DMAs
    alternated across the two HWDGE rings (sync/scalar).
  - Host unshards by rotating each slab back and stacking.
"""

import os as _os

_os.environ.setdefault("JAX_PLATFORMS", "axon,cpu")

import numpy as np

import concourse.bass as bass
import concourse.mybir as mybir
import concourse.tile as tile
from concourse import bacc
from concourse.bass_utils import run_bass_kernel_spmd

N = 8192          # nodes
F = 64            # contraction (K*D = 2*32)
NCORES = 8
SHARD = N // NCORES        # 1024 rows per core
HALF = N // 2              # 4096 local columns per PE row-group
MT = 128                   # output-row strip height
NT = 512                   # matmul moving free dim (one PSUM bank fp32)
DMA_CHUNK = 4096           # output DMA width in cols (1 MiB fp16 per dma_start)
DUAL_RING = True           # alternate output DMAs across sync/scalar HWDGE rings

F32 = mybir.dt.float32
F16 = mybir.dt.float16
NP_IN = np.float16

IN_DT = F16                # node/edge dtype on device (matmul inputs)
OUT_DT = F16               # strip + output DMA dtype


def build_nc():
    nc = bacc.Bacc("TRN2", target_bir_lowering=False, debug=False)

    node2_d = nc.dram_tensor("node2", [64, SHARD], IN_DT, kind="ExternalInput")
    edge2_d = nc.dram_tensor("edge2", [128, HALF], IN_DT, kind="ExternalInput")
    mask_d = nc.dram_tensor("dmask", [128, MT], OUT_DT, kind="ExternalInput")
    out_d = nc.dram_tensor("out", [SHARD, N], OUT_DT, kind="ExternalOutput")

    n_strips = SHARD // MT           # 8
    n_chunks = HALF // NT            # 8 matmul pairs per strip

    with tile.TileContext(nc) as tc:
        with (
            tc.tile_pool(name="consts", bufs=1) as consts,
            tc.tile_pool(name="outp", bufs=3) as outp,
            tc.tile_pool(name="psp", bufs=4, space=bass.MemorySpace.PSUM) as psp,
        ):
            node_sb = consts.tile([128, SHARD], IN_DT)
            mask_sb = consts.tile([128, MT], OUT_DT)
            edge_sb = consts.tile([128, HALF], IN_DT)

            # ordered so the bytes gating the first matmul pair land first:
            # edge chunk 0, node strip 0, mask, then the rest interleaved
            nodedst = node_sb[0:64, :]
            nc.sync.dma_start(out=edge_sb[:, 0:NT], in_=edge2_d[:, 0:NT])
            nc.sync.dma_start(out=nodedst[:, 0:MT], in_=node2_d[:, 0:MT])
            nc.sync.dma_start(out=mask_sb[:], in_=mask_d[:, :])
            for j in range(1, n_chunks):
                nc.sync.dma_start(
                    out=edge_sb[:, j * NT:(j + 1) * NT],
                    in_=edge2_d[:, j * NT:(j + 1) * NT],
                )
            nc.sync.dma_start(out=nodedst[:, MT:], in_=node2_d[:, MT:])
            nc.vector.tensor_copy(node_sb[64:128, 0:MT], node_sb[0:64, 0:MT])
            nc.vector.tensor_copy(node_sb[64:128, MT:], node_sb[0:64, MT:])

            for m in range(n_strips):
                strip = outp.tile([128, N], OUT_DT)
                lhs_lo = node_sb[0:64, m * MT:(m + 1) * MT]
                lhs_hi = node_sb[64:128, m * MT:(m + 1) * MT]
                for n in range(n_chunks):
                    ps_a = psp.tile([128, NT], F32)
                    ps_b = psp.tile([128, NT], F32)
                    nc.tensor.matmul(
                        ps_a[:],
                        lhs_lo,
                        edge_sb[0:64, n * NT:(n + 1) * NT],
                        start=True, stop=True,
                        tile_position=(0, 0),
                    )
                    nc.tensor.matmul(
                        ps_b[:],
                        lhs_hi,
                        edge_sb[64:128, n * NT:(n + 1) * NT],
                        start=True, stop=True,
                        tile_position=(64, 0),
                    )
                    nc.scalar.activation(
                        strip[:, n * NT:(n + 1) * NT], ps_a[:],
                        mybir.ActivationFunctionType.Relu,
                    )
                    nc.vector.tensor_scalar_max(
                        strip[:, HALF + n * NT:HALF + (n + 1) * NT], ps_b[:], 0.0,
                    )
                # zero the diagonal block (always local cols [m*MT, (m+1)*MT))
                nc.vector.tensor_mul(
                    strip[:, m * MT:(m + 1) * MT],
                    strip[:, m * MT:(m + 1) * MT],
                    mask_sb[:],
                )
                if m == 0:
                    # finer leading chunks: the first write starts after two
                    # matmul pairs instead of eight, shrinking the ramp gap
                    bounds = [0, 512, 1024, 2048, 4096, 8192]
                elif m == n_strips - 1:
                    # finer trailing chunks shrink the drain tail
                    bounds = [0, 4096, 6144, 7168, 8192]
                else:
                    bounds = list(range(0, N + 1, DMA_CHUNK))
                for q, (lo, hi) in enumerate(zip(bounds[:-1], bounds[1:])):
                    eng = nc.scalar if (DUAL_RING and q % 2 == 1) else nc.sync
                    eng.dma_start(
                        out=out_d[m * MT:(m + 1) * MT, lo:hi],
                        in_=strip[:, lo:hi],
                    )

    nc.compile()
    return nc


_NC = None


def _get_nc():
    global _NC
    if _NC is None:
        _NC = build_nc()
    return _NC


def make_in_maps(node_features: np.ndarray, edge_features: np.ndarray):
    node = np.ascontiguousarray(node_features, dtype=np.float32).reshape(N, F)
    edge = np.ascontiguousarray(edge_features, dtype=np.float32).reshape(N, F)
    edge_t = np.ascontiguousarray(edge.T.astype(NP_IN))            # [64, 8192]
    mask = np.ones((128, MT), NP_IN)
    np.fill_diagonal(mask, 0.0)

    in_maps = []
    for c in range(NCORES):
        node_t = node[c * SHARD:(c + 1) * SHARD].T.astype(NP_IN)   # [64, 1024]
        node2 = np.ascontiguousarray(node_t)
        et = np.roll(edge_t, -c * SHARD, axis=1)       # local col j' = global (j'+c*1024)%N
        edge2 = np.ascontiguousarray(np.concatenate([et[:, :HALF], et[:, HALF:]], axis=0))
        in_maps.append({"node2": node2, "edge2": edge2, "dmask": mask})
    return in_maps


def kernel(node_features: np.ndarray, edge_features: np.ndarray) -> np.ndarray:
    nc = _get_nc()
    in_maps = make_in_maps(node_features, edge_features)
    res = run_bass_kernel_spmd(nc, in_maps, core_ids=list(range(NCORES)))
    out = np.empty((N, N), np.float32)
    for c in range(NCORES):
        out[c * SHARD:(c + 1) * SHARD] = np.roll(res.results[c]["out"], c * SHARD, axis=1)
    return out


# revision 4
# speedup vs baseline: 1.0131x; 1.0131x over previous
"""ContextWeaver: context[i, j] = relu(sum_{k,d} node[i,k,d] * edge[j,k,d]), diag zeroed.

Strategy (8 NeuronCores, SPMD):
  - Shard node rows 8-way (1024 rows/core); replicate edge^T per core with a
    per-core column rotation of c*1024 so every core runs the identical
    instruction stream. The diagonal is zeroed on the host during unshard
    (np.fill_diagonal) -- no mask tensor or extra vector op on device.
  - Contraction dim is 64 (= K*D); pack two independent 64-row matmuls into
    the 128x128 PE array with tile_position row tiling: partitions 0-63
    compute local columns [0, 4096), partitions 64-127 compute [4096, 8192).
  - The kernel is output-DMA bound (256 MB result). Inputs and outputs are
    fp16 on device: matmul in fp16 (fp32 PSUM accumulate), relu converts
    PSUM fp32 -> fp16 strips, output DMA moves half the bytes of fp32. The
    host upcasts to fp32 on unshard; total rounding error ~5e-4 relative,
    far inside the 2e-2 gate.
  - PSUM tiles are [128, 1024] (2 banks, 2 matmuls each) so each relu
    instruction covers 1024 columns, amortizing per-instruction overhead;
    ScalarE relus the low half, VectorE the high half concurrently.
  - 512 KiB output DMAs alternate across the two HWDGE rings (sync/scalar)
    in data-readiness order; input loads are 3 issues split across both
    rings (HWDGE rings are FIFO, so output issues must not queue behind
    input). First/last strips use finer chunks to shrink ramp and tail.
  - Host unshards by rotating each slab back, stacking, upcasting.
"""

import os as _os

_os.environ.setdefault("JAX_PLATFORMS", "axon,cpu")

import numpy as np

import concourse.bass as bass
import concourse.mybir as mybir
import concourse.tile as tile
from concourse import bacc
from concourse.bass_utils import run_bass_kernel_spmd

N = 8192          # nodes
F = 64            # contraction (K*D = 2*32)
NCORES = 8
SHARD = N // NCORES        # 1024 rows per core
HALF = N // 2              # 4096 local columns per PE row-group
MT = 128                   # output-row strip height
NT = 512                   # matmul moving free dim (one PSUM bank fp32)
GT = 2 * NT                # relu granularity / PSUM tile width (2 banks)

F32 = mybir.dt.float32
F16 = mybir.dt.float16
NP_IN = np.float16

IN_DT = F16                # node/edge dtype on device (matmul inputs)
OUT_DT = F16               # strip + output DMA dtype


def build_nc():
    nc = bacc.Bacc("TRN2", target_bir_lowering=False, debug=False)

    node2_d = nc.dram_tensor("node2", [64, SHARD], IN_DT, kind="ExternalInput")
    edge2_d = nc.dram_tensor("edge2", [128, HALF], IN_DT, kind="ExternalInput")
    out_d = nc.dram_tensor("out", [SHARD, N], OUT_DT, kind="ExternalOutput")

    n_strips = SHARD // MT           # 8
    n_gens = HALF // GT              # 4 psum generations per strip

    with tile.TileContext(nc) as tc:
        with (
            tc.tile_pool(name="consts", bufs=1) as consts,
            tc.tile_pool(name="outp", bufs=4) as outp,
            tc.tile_pool(name="psp", bufs=2, space=bass.MemorySpace.PSUM) as psp,
        ):
            node_sb = consts.tile([128, SHARD], IN_DT)
            edge_sb = consts.tile([128, HALF], IN_DT)

            # 3 input issues split across the FIFO HWDGE rings: bytes gating
            # the first matmuls (edge cols 0:1024 + node) on sync, rest on
            # scalar so they don't delay sync's first output issue.
            nc.sync.dma_start(out=edge_sb[:, 0:GT], in_=edge2_d[:, 0:GT])
            nc.sync.dma_start(out=node_sb[0:64, :], in_=node2_d[:])
            nc.scalar.dma_start(out=edge_sb[:, GT:], in_=edge2_d[:, GT:])
            nc.vector.tensor_copy(node_sb[64:128, :], node_sb[0:64, :])

            for m in range(n_strips):
                strip = outp.tile([128, N], OUT_DT)
                lhs_lo = node_sb[0:64, m * MT:(m + 1) * MT]
                lhs_hi = node_sb[64:128, m * MT:(m + 1) * MT]
                for t in range(n_gens):
                    ps_lo = psp.tile([128, GT], F32)
                    ps_hi = psp.tile([128, GT], F32)
                    for h in range(2):
                        nc.tensor.matmul(
                            ps_lo[:, h * NT:(h + 1) * NT],
                            lhs_lo,
                            edge_sb[0:64, t * GT + h * NT:t * GT + (h + 1) * NT],
                            start=True, stop=True,
                            tile_position=(0, 0),
                        )
                    for h in range(2):
                        nc.tensor.matmul(
                            ps_hi[:, h * NT:(h + 1) * NT],
                            lhs_hi,
                            edge_sb[64:128, t * GT + h * NT:t * GT + (h + 1) * NT],
                            start=True, stop=True,
                            tile_position=(64, 0),
                        )
                    nc.scalar.activation(
                        strip[:, t * GT:(t + 1) * GT], ps_lo[:],
                        mybir.ActivationFunctionType.Relu,
                    )
                    nc.vector.tensor_scalar_max(
                        strip[:, HALF + t * GT:HALF + (t + 1) * GT], ps_hi[:], 0.0,
                    )
                # (lo, hi, ring) in data-readiness order; rings alternate so
                # both HWDGE queues stay fed. Low half ready after scalar
                # relus, high half after vector relus, interleaved by gen.
                if m == 0:
                    chunks = [(0, 1024, 0), (HALF, HALF + 1024, 1),
                              (1024, 2048, 0), (HALF + 1024, HALF + 2048, 1),
                              (2048, HALF, 0), (HALF + 2048, N, 1)]
                elif m == n_strips - 1:
                    chunks = []
                    for t in range(4):
                        chunks.append((t * 1024, (t + 1) * 1024, 0))
                        chunks.append((HALF + t * 1024, HALF + (t + 1) * 1024, 1))
                else:
                    chunks = [(0, 2048, 0), (HALF, HALF + 2048, 1),
                              (2048, HALF, 0), (HALF + 2048, N, 1)]
                for lo, hi, ring in chunks:
                    eng = nc.scalar if ring else nc.sync
                    eng.dma_start(
                        out=out_d[m * MT:(m + 1) * MT, lo:hi],
                        in_=strip[:, lo:hi],
                    )

    nc.compile()
    return nc


_NC = None


def _get_nc():
    global _NC
    if _NC is None:
        _NC = build_nc()
    return _NC


def make_in_maps(node_features: np.ndarray, edge_features: np.ndarray):
    node = np.ascontiguousarray(node_features, dtype=np.float32).reshape(N, F)
    edge = np.ascontiguousarray(edge_features, dtype=np.float32).reshape(N, F)
    edge_t = np.ascontiguousarray(edge.T.astype(NP_IN))            # [64, 8192]

    in_maps = []
    for c in range(NCORES):
        node_t = node[c * SHARD:(c + 1) * SHARD].T.astype(NP_IN)   # [64, 1024]
        node2 = np.ascontiguousarray(node_t)
        et = np.roll(edge_t, -c * SHARD, axis=1)       # local col j' = global (j'+c*1024)%N
        edge2 = np.ascontiguousarray(np.concatenate([et[:, :HALF], et[:, HALF:]], axis=0))
        in_maps.append({"node2": node2, "edge2": edge2})
    return in_maps


def kernel(node_features: np.ndarray, edge_features: np.ndarray) -> np.ndarray:
    nc = _get_nc()
    in_maps = make_in_maps(node_features, edge_features)
    res = run_bass_kernel_spmd(nc, in_maps, core_ids=list(range(NCORES)))
    out = np.empty((N, N), np.float32)
    for c in range(NCORES):
        out[c * SHARD:(c + 1) * SHARD] = np.roll(res.results[c]["out"], c * SHARD, axis=1)
    np.fill_diagonal(out, 0.0)
    return out
